# revision 2
# baseline (speedup 1.0000x reference)
"""Trainium2 Bass kernel v3 for nn_Correlation (FlowNet-style cost volume).

out[b, 21*di+dj, h, w] = leaky_0.1((1/256) sum_c in1[b,c,h,w] *
                                   in2pad[b,c,h+2di,w+2dj]), pad 20.

Per core = 1 sample (data-parallel over B across 8 cores).

Pipeline per "band" (he-block a in [0,6) x parity (hp,wp) x we-block wb in
[0,4); pixel block = 8he x 16we = 128, p = he*16+we):
  1. PE: psum[pixel, (i,u)] over win rows i in [0,28), cols u in [0,36),
     contracting C=256 (bf16); moving operand streams straight from a
     zero-padded bf16 full-image staging via 3D strided APs (no
     deinterleave); in1 pixel blocks prebuilt per block-row.
  2. Evac psum -> band_sb bf16 [128, 1008] with 1/C folded in (DVE+ACT).
  3. he-shear on GPSIMD via indirect_copy (group-of-16-uniform index):
     B2[p, v] = band_sb[p, 36*(p//16) + v]   (v in [0, 756))
  4. Spill B2 -> DRAM (1 DMA); gather back the we-shear diagonal
     (DRAM-side diag legal, 3D AP, 1 DMA): alig[p, t] = B2d[p, p%16 + t].
  5. DVE compact: alig_c[p, 21*di+dj] = alig[p, 36*di+dj]  [128, 441].
  6. 4 PE transposes (bf16) -> tr[d_local, pixel] in PSUM.
  7. One fused leaky: out_e = max(x, 0.1x) with 4D strided dst (DVE).
  8. Per block-row: 4 casting stores (bf16->f32), d-contiguous.
"""

import numpy as np

import concourse.bass as bass
import concourse.mybir as mybir
from concourse.tile import TileContext
from concourse.bass_utils import run_bass_kernel_spmd
from concourse.masks import make_identity

DT = mybir.dt

B, C, H, W = 8, 256, 96, 128
NP = 21
ND = NP * NP
CC = 2
HE, WE = H // 2, W // 2

HB, WBK = 8, 16              # pixel block: p = he*16 + we
NWB = WE // WBK              # 4
NA = HE // HB                # 6
WIN_I, WIN_U = HB + 20, WBK + 20   # 28 x 36
PITCH = WIN_U                # 36
FB = WIN_I * WIN_U           # 1008
HW = H * W

SROWS = H + 40
STG2_F = SROWS * W           # 17408
STG1_F = 2 * HB * W          # 2048 (16 full-res rows)
BLK_F = CC * 4 * NWB * 128   # 4096

B2W = (WBK - 1) + 20 * PITCH + NP   # 756
PAIRW = PITCH + B2W                 # 792: he-pair spill span
SPAN = 20 * PITCH + NP       # 741
APAD = 768
ACW = 448                    # compact width (441 + pad)
OUT_F = 4 * STG1_F           # 8192

_MAX_WAITS = 1


def _split_excess_waits(nc):
    """This walrus build accepts only ONE sync-wait per instruction; Tile
    emits multi-waits. Hoist excess waits onto same-engine NOPs inserted
    right before the over-subscribed instruction."""
    nid = 0
    for f in nc.m.functions:
        for blk in f.blocks:
            insts = list(blk.instructions)
            out = []
            changed = False
            for inst in insts:
                si = inst.sync_info
                if si is not None and si.on_wait and len(si.on_wait) > _MAX_WAITS:
                    waits = list(si.on_wait)
                    extra, keep = waits[:-_MAX_WAITS], waits[-_MAX_WAITS:]
                    for k in range(0, len(extra), _MAX_WAITS):
                        nop = mybir.InstNoOp(name=f"I-waitsplit-{nid}", ins=[], outs=[])
                        nid += 1
                        nop.engine = inst.engine
                        nop.sync_info = mybir.SyncInfo(
                            on_wait=extra[k : k + _MAX_WAITS], on_update=[]
                        )
                        out.append(nop)
                        changed = True
                    si.on_wait = keep
                    inst.sync_info = si
                out.append(inst)
            if changed:
                blk.instructions = out
    return nc


def _ap(t, off_extra, dims):
    return bass.AP(tensor=t.tensor, offset=t.offset + off_extra, ap=dims)


DBG_BAND = (0, 0, 0, 1)


def _build_nc(debug=False):
    nc = bass.Bass()
    in1_d = nc.dram_tensor("in1", [C, H, W], DT.float32, kind="ExternalInput")
    in2_d = nc.dram_tensor("in2", [C, H, W], DT.float32, kind="ExternalInput")
    out_d = nc.dram_tensor("out", [ND, H, W], DT.float32, kind="ExternalOutput")
    dbg = {}
    if debug:
        for nm, shp in [
            ("band_sb", [128, FB]), ("b2", [128, B2W]),
            ("alig", [128, APAD]), ("aligc", [128, ACW]),
            ("out_e", [128, OUT_F]),
        ]:
            dbg[nm] = nc.dram_tensor(
                f"dbg_{nm}", shp, DT.bfloat16, kind="ExternalOutput")

    def _dump(nm, tile, fsz):
        if debug:
            nc.sync.dma_start(
                bass.AP(tensor=dbg[nm], offset=0, ap=[[fsz, 128], [1, fsz]]),
                tile[:, 0:fsz],
            )

    with TileContext(nc) as tc:
        with (
            tc.tile_pool(name="constp", bufs=1) as constp,
            tc.tile_pool(name="stg2p", bufs=1) as stg2p,
            tc.tile_pool(name="stg1p", bufs=2) as stg1p,
            tc.tile_pool(name="in1bp", bufs=2) as in1bp,
            tc.tile_pool(name="bsbp", bufs=3) as bsbp,
            tc.tile_pool(name="b2p", bufs=3) as b2p,
            tc.tile_pool(name="aligp", bufs=3) as aligp,
            tc.tile_pool(name="aligcp", bufs=3) as aligcp,
            tc.tile_pool(name="relup", bufs=2) as relup,
            tc.tile_pool(name="outep", bufs=1) as outep,
            tc.tile_pool(name="psp", bufs=3, space="PSUM") as psp,
            tc.tile_pool(name="trpp", bufs=2, space="PSUM") as trpp,
            tc.tile_pool(name="dramp", bufs=4, space="DRAM") as dramp,
        ):
            identity = constp.tile([128, 128], DT.bfloat16)
            make_identity(nc, identity)


            stg2 = [stg2p.tile([128, STG2_F], DT.bfloat16, name=f"stg2_{cc}")
                    for cc in range(CC)]
            for cc in range(CC):
                nc.vector.memset(stg2[cc][:, 0 : 20 * W], 0.0)
                nc.vector.memset(stg2[cc][:, (H + 20) * W : SROWS * W], 0.0)
                nc.gpsimd.dma_start(
                    _ap(stg2[cc], 20 * W, [[STG2_F, 128], [1, H * W]]),
                    in2_d[cc * 128 : (cc + 1) * 128, :, :],
                )

            def load_stg1(a):
                tiles = []
                for cc in range(CC):
                    t = stg1p.tile([128, STG1_F], DT.bfloat16, name=f"stg1_{cc}")
                    nc.gpsimd.dma_start(
                        t[:, :],
                        in1_d[cc * 128 : (cc + 1) * 128,
                              2 * HB * a : 2 * HB * (a + 1), :],
                    )
                    tiles.append(t)
                return tiles

            def build_in1blk(stg1):
                blk = in1bp.tile([128, BLK_F], DT.bfloat16, name="in1blk")
                k = 0
                for cc in range(CC):
                    for hp in range(2):
                        for wp in range(2):
                            src = _ap(
                                stg1[cc], hp * W + wp,
                                [[STG1_F, 128], [2 * WBK, NWB],
                                 [2 * W, HB], [2, WBK]],
                            )
                            dst = _ap(
                                blk, ((cc * 2 + hp) * 2 + wp) * NWB * 128,
                                [[BLK_F, 128], [128, NWB], [WBK, HB], [1, WBK]],
                            )
                            if k % 4 < 2:
                                nc.vector.tensor_copy(dst, src)
                            else:
                                nc.scalar.copy(dst, src)
                            k += 1
                return blk

            stg1 = load_stg1(0)
            in1blk = build_in1blk(stg1)
            # staging init (zero pads, casts, iota table, in1 block) must
            # complete before the first band's matmuls/indirect reads
            nc.all_engine_barrier()

            for a in range(NA):
                he0 = HB * a
                out_e = outep.tile([128, OUT_F], DT.bfloat16, name="out_e")
                for hp in range(2):
                    for wp in range(2):
                        for wb in range(NWB):
                            u0 = max(0, 10 - WBK * wb)
                            u1 = min(WIN_U, 10 + WE - WBK * wb)
                            nu = u1 - u0
                            ps = [
                                psp.tile([128, 504], DT.float32, name="ps_a"),
                                psp.tile([128, 504], DT.float32, name="ps_b"),
                            ]
                            for b in range(2):
                                for cc in range(CC):
                                    lh = _ap(
                                        in1blk,
                                        (((cc * 2 + hp) * 2 + wp) * NWB + wb)
                                        * 128,
                                        [[BLK_F, 128], [1, 128]],
                                    )
                                    i0 = 14 * b
                                    roff = (2 * (he0 + i0) + hp) * W \
                                        + 2 * (WBK * wb + u0 - 10) + wp
                                    rhs = _ap(
                                        stg2[cc], roff,
                                        [[STG2_F, 128], [2 * W, 14], [2, nu]],
                                    )
                                    nc.tensor.matmul(
                                        ps[b][:, 0 : 14 * nu],
                                        lh,
                                        rhs,
                                        start=(cc == 0),
                                        stop=(cc == CC - 1),
                                    )
                            band_sb = bsbp.tile([128, FB], DT.bfloat16,
                                                name="band_sb")
                            if u0 > 0:
                                nc.vector.memset(
                                    _ap(band_sb, 0,
                                        [[FB, 128], [PITCH, WIN_I], [1, u0]]),
                                    0.0,
                                )
                            if u1 < WIN_U:
                                nc.vector.memset(
                                    _ap(band_sb, u1,
                                        [[FB, 128], [PITCH, WIN_I],
                                         [1, WIN_U - u1]]),
                                    0.0,
                                )
                            for b in range(2):
                                src = _ap(ps[b], 0, [[504, 128], [1, 14 * nu]])
                                if nu == WIN_U:
                                    dst = _ap(band_sb, 504 * b,
                                              [[FB, 128], [1, 14 * nu]])
                                else:
                                    dst = _ap(band_sb, 504 * b + u0,
                                              [[FB, 128], [PITCH, 14], [1, nu]])
                                if b == 0:
                                    nc.vector.tensor_scalar(
                                        dst, src, 1.0 / C, None,
                                        mybir.AluOpType.mult)
                                else:
                                    nc.scalar.mul(dst, src, 1.0 / C)
                            if debug and (a, hp, wp, wb) == DBG_BAND:
                                _dump("band_sb", band_sb, FB)
                            # spill per-he-pair compact spans, then gather the
                            # full (he,we) diagonal back (DRAM-side diag)
                            bdram = dramp.tile([128, PAIRW], DT.bfloat16,
                                               name="bdram")
                            for k in range(4):
                                nc.sync.dma_start(
                                    _ap(bdram, 32 * k * PAIRW,
                                        [[PAIRW, 32], [1, PAIRW]]),
                                    _ap(band_sb, 32 * k * FB + 72 * k,
                                        [[FB, 32], [1, PAIRW]]),
                                )
                            alig = aligp.tile([128, APAD], DT.bfloat16,
                                              name="alig")
                            for k in range(4):
                                gsrc = _ap(
                                    bdram, 32 * k * PAIRW,
                                    [[16 * PAIRW + PITCH, 2],
                                     [PAIRW + 1, 16], [1, SPAN]],
                                )
                                gdst = _ap(alig, 32 * k * APAD,
                                           [[APAD, 32], [1, SPAN]])
                                nc.scalar.dma_start(gdst, gsrc)
                            if debug and (a, hp, wp, wb) == DBG_BAND:
                                _dump("alig", alig, APAD)
                            # compact 36-pitch -> 21-pitch
                            aligc = aligcp.tile([128, ACW], DT.bfloat16,
                                                name="aligc")
                            nc.vector.tensor_copy(
                                _ap(aligc, 0, [[ACW, 128], [1, ND]]),
                                _ap(alig, 0, [[APAD, 128], [PITCH, NP], [1, NP]]),
                            )
                            if debug and (a, hp, wp, wb) == DBG_BAND:
                                _dump("aligc", aligc, ACW)
                            # 4 PE transposes -> tr psum bf16 [d_local, pix]
                            tr = trpp.tile([128, 512], DT.bfloat16, name="tr")
                            for c in range(4):
                                nd = min(128, ND - 128 * c)
                                nc.tensor.transpose(
                                    tr[0:nd, 128 * c : 128 * c + 128],
                                    aligc[:, 128 * c : 128 * c + nd],
                                    identity[:, :],
                                )
                            # leaky(x) = 0.1x + relu(0.9x): scalar Relu then
                            # vector stt (one PSUM operand each)
                            relu_sb = relup.tile([128, 512], DT.bfloat16,
                                                 name="relu_sb")
                            nc.scalar.activation(
                                relu_sb[:, :], tr[:, :],
                                mybir.ActivationFunctionType.Relu,
                                bias=0.0, scale=0.9,
                            )
                            src = _ap(tr, 0,
                                      [[512, 128], [128, 4], [WBK, HB],
                                       [1, WBK]])
                            rsrc = _ap(relu_sb, 0,
                                       [[512, 128], [128, 4], [WBK, HB],
                                        [1, WBK]])
                            dst = _ap(out_e, hp * W + 2 * WBK * wb + wp,
                                      [[OUT_F, 128], [STG1_F, 4],
                                       [2 * W, HB], [2, WBK]])
                            nc.vector.scalar_tensor_tensor(
                                dst, src, 0.1, rsrc,
                                mybir.AluOpType.mult, mybir.AluOpType.add,
                            )
                if a + 1 < NA:
                    stg1 = load_stg1(a + 1)
                    in1blk = build_in1blk(stg1)
                if debug and a == 0:
                    _dump("out_e", out_e, OUT_F)
                for c in range(4):
                    nd = min(128, ND - 128 * c)
                    nc.gpsimd.dma_start(
                        bass.AP(
                            tensor=out_d,
                            offset=128 * c * HW + 2 * HB * a * W,
                            ap=[[HW, nd], [1, STG1_F]],
                        ),
                        out_e[0:nd, c * STG1_F : (c + 1) * STG1_F],
                    )

    return _split_excess_waits(nc)


_NC_CACHE = None


def _get_nc():
    global _NC_CACHE
    if _NC_CACHE is None:
        _NC_CACHE = _build_nc()
    return _NC_CACHE


def kernel(input1, input2):
    input1 = np.ascontiguousarray(np.asarray(input1, dtype=np.float32))
    input2 = np.ascontiguousarray(np.asarray(input2, dtype=np.float32))
    assert input1.shape == (B, C, H, W) and input2.shape == (B, C, H, W)
    nc = _get_nc()
    in_maps = [{"in1": input1[b], "in2": input2[b]} for b in range(B)]
    res = run_bass_kernel_spmd(nc, in_maps, core_ids=list(range(B)))
    return np.stack([res.results[b]["out"] for b in range(B)], axis=0)
